# revision 8
# baseline (speedup 1.0000x reference)
"""Trainium2 Bass kernel for SimCLR-style contrastive loss (NT-Xent).

Reference computation (B=4096, D=128, fp32):
    zi = z_i / ||z_i||, zj = z_j / ||z_j||, reps = concat([zi, zj])  # (8192, 128)
    sim = (reps @ reps.T) / 0.5                                      # (8192, 8192)
    pos[i] = sim[i, (i + 4096) % 8192]
    lse[i] = logsumexp(sim[i, :] with diagonal masked to -inf)
    loss = mean(lse - pos)

Sharding: data-parallel over the 8192 rows -> 1024 rows per core, with the
full 8192-row column set replicated per core.  To keep the program uniform
SPMD, each core receives a copy of the raw concatenated input *rolled* so
that its own 1024 rows sit at local rows 0..1023.  Then for every core:
  - local row r == local column r            (diagonal/self entry)
  - positive for local row r is local column (r + 4096) % 8192
so diag/pos extraction offsets are core-independent.

Per-core device program:
  1. Load rolled (8192, 128) fp32, 64 tiles of [128 rows, 128 feat].
  2. Row sumsq on DVE (tensor_tensor_reduce), rsqrt = exp(-0.5*ln(x)) on ACT
     (Ln and Exp live in the same activation-table set -> one table load).
  3. Scale rows by rsqrt on DVE -> fp16, transpose via PE into
     repsT[128 feat, 8192 rows] (16 chunks of [128, 512] fp16).
  4. For each 1024-col chunk n (8) x row tile t (8): two N=512 fp16 matmuls
     into one [128, 1024] PSUM tile (2 banks), then one ACT Exp(scale=2)
     over the 1024 columns with accum_out -> per-row partial sums.
     On chunk n==0 extract diagonal sim values, on n==4 the positives
     (multiply with an eye mask + reduce on DVE, straight from PSUM).
  5. lse = Ln(S_total - Exp(2*diag)); contrib = lse - 2*pos; reduce 1024
     rows to a single scalar via a ones-vector matmul; DMA out [1,1] fp32.

Host: loss = sum(core partials) / 8192.

No cross-core communication: the "all-reduce" of the 8 partial scalars is
the host-side gather/unshard step.
"""

import os
import sys
import numpy as np
from contextlib import ExitStack

for _p in ("/opt/trn_rl_repo",):
    if _p not in sys.path and os.path.isdir(_p):
        sys.path.insert(0, _p)

import concourse.bass as bass  # noqa: E402
import concourse.bacc as bacc  # noqa: E402
import concourse.mybir as mybir  # noqa: E402
import concourse.tile as tile  # noqa: E402
from concourse import bass_utils  # noqa: E402

B = 4096
D = 128
N = 2 * B  # 8192 total rows
NCORES = 8
ROWS = N // NCORES  # 1024 rows per core
RT = ROWS // 128  # 8 row tiles per core
NK = N // 128  # 64 column tiles of 128 rows each
NCH512 = N // 512  # 16 repsT chunks of 512
NCH = N // 1024  # 8 matmul/exp column chunks of 1024

F32 = mybir.dt.float32
F16 = mybir.dt.float16
AF = mybir.ActivationFunctionType
OP = mybir.AluOpType
AX = mybir.AxisListType


def _trace_kernel(ctx, tc, cols, ident, eye, ones, out):
    nc = tc.nc

    const_pool = ctx.enter_context(tc.tile_pool(name="const", bufs=1))
    raw_pool = ctx.enter_context(tc.tile_pool(name="raw", bufs=10))
    nrm_pool = ctx.enter_context(tc.tile_pool(name="nrm", bufs=4))
    sq_pool = ctx.enter_context(tc.tile_pool(name="sq", bufs=2))
    stat_pool = ctx.enter_context(tc.tile_pool(name="stat", bufs=1))
    repsT_pool = ctx.enter_context(tc.tile_pool(name="repsT", bufs=1))
    exps_pool = ctx.enter_context(tc.tile_pool(name="exps", bufs=2))
    dp_pool = ctx.enter_context(tc.tile_pool(name="dp", bufs=2))
    tpsum_pool = ctx.enter_context(tc.tile_pool(name="tpsum", bufs=1, space="PSUM"))
    mpsum_pool = ctx.enter_context(tc.tile_pool(name="mpsum", bufs=6, space="PSUM"))
    fpsum_pool = ctx.enter_context(tc.tile_pool(name="fpsum", bufs=1, space="PSUM"))

    identity = const_pool.tile([128, 128], F16, name="identity")
    nc.sync.dma_start(out=identity[:], in_=ident)
    eyemask = const_pool.tile([128, 128], F32, name="eyemask")
    nc.sync.dma_start(out=eyemask[:], in_=eye)
    ones_t = const_pool.tile([128, 1], F32, name="ones_t")
    nc.sync.dma_start(out=ones_t[:], in_=ones)

    sumsq = stat_pool.tile([128, NK], F32, name="sumsq")
    rln = stat_pool.tile([128, NK], F32, name="rln")
    rsq = stat_pool.tile([128, NK], F32, name="rsq")

    # 16 persistent fp16 chunks [128 feat, 512 rows] holding reps.T
    repsT = [
        repsT_pool.tile([128, 512], F16, name=f"repsT{i}", tag=f"repsT{i}")
        for i in range(NCH512)
    ]

    # ---- Phase 1: load, normalize, transpose ----
    GROUP = 8  # rsqrt batch size (column tiles)
    tp = None
    for g in range(NK // GROUP):
        raws = []
        sqg = sq_pool.tile([128, GROUP, D], F32, tag="sqg", name=f"sqg{g}")
        for j in range(GROUP):
            k = g * GROUP + j
            raw = raw_pool.tile([128, D], F32, tag="raw", name=f"raw{k}")
            nc.sync.dma_start(out=raw[:], in_=cols[k * 128:(k + 1) * 128, :])
            nc.vector.tensor_mul(sqg[:, j, :], raw[:], raw[:])
            raws.append((k, raw))
        gs = slice(g * GROUP, (g + 1) * GROUP)
        nc.vector.tensor_reduce(out=sumsq[:, gs], in_=sqg[:], axis=AX.X, op=OP.add)
        nc.scalar.activation(rln[:, gs], sumsq[:, gs], AF.Ln)
        nc.scalar.activation(rsq[:, gs], rln[:, gs], AF.Exp, scale=-0.5)
        for (k, raw) in raws:
            nrm = nrm_pool.tile([128, D], F16, tag="nrm", name=f"nrm{k}")
            nc.vector.tensor_scalar_mul(nrm[:], raw[:], rsq[:, k:k + 1])
            if k % 4 == 0:
                tp = tpsum_pool.tile([128, 512], F16, tag="tp", name=f"tp{k // 4}")
            q = k % 4
            nc.tensor.transpose(tp[:, q * 128:(q + 1) * 128], nrm[:], identity[:])
            if k % 4 == 3:
                nc.vector.tensor_copy(repsT[k // 4][:], tp[:])

    # ---- Phase 2: similarity chunks + exp row-sums + diag/pos ----
    # sums_t[t][:, n] = sum over 512-col chunk n of exp(2*sim) for row tile t
    sums_t = [
        stat_pool.tile([128, NCH512], F32, name=f"sums{t}") for t in range(RT)
    ]
    dpos = stat_pool.tile([128, 2 * RT], F32, name="dpos")  # [diag x8 | pos x8]
    for n in range(NCH512):
        for t in range(RT):
            mp = mpsum_pool.tile([128, 512], F32, tag="mp", name=f"mp{n}_{t}")
            lhsT = repsT[t // 4][:, (t % 4) * 128:(t % 4 + 1) * 128]
            nc.tensor.matmul(mp[:], lhsT, repsT[n][:], start=True, stop=True)
            es = exps_pool.tile([128, 512], F16, tag="es", name=f"es{n}_{t}")
            nc.scalar.activation(
                es[:], mp[:], AF.Exp, scale=2.0, accum_out=sums_t[t][:, n:n + 1],
            )
            if n == t // 4 or n == 8 + t // 4:
                off = (t % 4) * 128
                scr = dp_pool.tile([128, 128], F32, tag="scr", name=f"scr{n}_{t}")
                col = t if n == t // 4 else RT + t
                nc.vector.tensor_mul(scr[:], mp[:, off:off + 128], eyemask[:])
                nc.vector.tensor_reduce(
                    out=dpos[:, col:col + 1], in_=scr[:], axis=AX.X, op=OP.add
                )

    # ---- Phase 3: lse and reduction ----
    salls = stat_pool.tile([128, RT], F32, name="salls")
    for t in range(RT):
        nc.vector.tensor_reduce(
            out=salls[:, t:t + 1], in_=sums_t[t][:], axis=AX.X, op=OP.add
        )
    ed = stat_pool.tile([128, RT], F32, name="ed")
    nc.scalar.activation(ed[:], dpos[:, 0:RT], AF.Exp, scale=2.0)
    snd = stat_pool.tile([128, RT], F32, name="snd")
    nc.vector.tensor_sub(snd[:], salls[:], ed[:])
    lse = stat_pool.tile([128, RT], F32, name="lse")
    nc.scalar.activation(lse[:], snd[:], AF.Ln)
    negp = stat_pool.tile([128, RT], F32, name="negp")
    nc.vector.tensor_scalar_mul(negp[:], dpos[:, RT:2 * RT], -2.0)
    contrib = stat_pool.tile([128, RT], F32, name="contrib")
    nc.vector.tensor_add(contrib[:], lse[:], negp[:])
    tot = stat_pool.tile([128, 1], F32, name="tot")
    nc.vector.tensor_reduce(out=tot[:], in_=contrib[:], axis=AX.X, op=OP.add)

    fp = fpsum_pool.tile([1, 1], F32, name="fp")
    nc.tensor.matmul(fp[:], tot[:], ones_t[:], start=True, stop=True)
    res = stat_pool.tile([1, 1], F32, name="res")
    nc.vector.tensor_copy(res[:], fp[:])
    nc.sync.dma_start(out=out, in_=res[:])


def build_nc():
    nc = bacc.Bacc("TRN2", debug=False, enable_asserts=False)
    cols = nc.dram_tensor("cols", (N, D), F32, kind="ExternalInput")
    ident = nc.dram_tensor("ident", (128, 128), F16, kind="ExternalInput")
    eye = nc.dram_tensor("eye32", (128, 128), F32, kind="ExternalInput")
    ones = nc.dram_tensor("ones", (128, 1), F32, kind="ExternalInput")
    out = nc.dram_tensor("partial", (1, 1), F32, kind="ExternalOutput")
    with tile.TileContext(nc) as tc, ExitStack() as ctx:
        _trace_kernel(ctx, tc, cols.ap(), ident.ap(), eye.ap(), ones.ap(), out.ap())
    nc.compile()
    return nc


_NC_CACHE = None


def _get_nc():
    global _NC_CACHE
    if _NC_CACHE is None:
        _NC_CACHE = build_nc()
    return _NC_CACHE


def make_in_maps(z_i, z_j):
    reps = np.concatenate(
        [np.asarray(z_i, np.float32), np.asarray(z_j, np.float32)], axis=0
    )
    ident = np.eye(128, dtype=np.float16)
    eye32 = np.eye(128, dtype=np.float32)
    ones = np.ones((128, 1), dtype=np.float32)
    return [
        {
            "cols": np.ascontiguousarray(np.roll(reps, -ROWS * c, axis=0)),
            "ident": ident,
            "eye32": eye32,
            "ones": ones,
        }
        for c in range(NCORES)
    ]


def run_on_hw(in_maps, trace=False, **kwargs):
    nc = _get_nc()
    return bass_utils.run_bass_kernel_spmd(
        nc, in_maps, core_ids=list(range(NCORES)), trace=trace, **kwargs
    )


def kernel(z_i, z_j):
    res = run_on_hw(make_in_maps(z_i, z_j))
    total = sum(float(r["partial"][0, 0]) for r in res.results)
    return np.array(total / N, dtype=np.float32)
